# revision 4
# baseline (speedup 1.0000x reference)
"""Trainium2 Bass kernel for grouped-expert 3-layer MLP (MoE, known covariance).

Computes, for x[B, E, DIN] and per-expert weights:
    h1 = relu(x[:,e] @ W1[e] + b1[e])      # [B, H]
    h2 = relu(h1 @ W2[e] + b2[e])          # [B, H]
    o  = h2 @ W3[e] + b3[e]                # [B, DOUT]
    out = sum_e o                          # [B, DOUT]

Sharding: data-parallel over batch across 8 NeuronCores (B=8192 -> 1024/core).
Weights are replicated to every core; no collectives needed.

Per-core schedule (v2):
  - All matmuls run in bf16 (1 cycle/row on the PE; fp8 would be 2x but its
    quantization noise measures ~3-6e-2 on this problem vs the 2e-2 gate).
  - x is staged once through a bf16 DRAM scratch (gpsimd cast DMA), then each
    (block, expert) slice is transposed straight into feature-major SBUF tiles
    by the hardware-DGE XBAR transpose DMA -- the PE does no x transposes.
  - Batch blocks (2 x 512) are the OUTER loop; all 16 experts' weights stay
    resident in SBUF (~88KB/partition) so block 1 re-uses them without DMA.
  - Layer 3 (M=DOUT=64) runs column-tiled: gb even -> PSUM partitions 0:64,
    gb odd -> 64:128, two concurrent matmuls in separate PE column groups.
    The two halves accumulate over all experts and are summed in the epilogue.
  - L3 of expert e is emitted after L1 of expert e+1 (software stagger) so its
    h2-evacuation dependency is met without stalling the PE.
  - A short burst of identity matmuls at t=0 warms the PE HAM clock gate
    (cold PE runs at 1.2GHz for the first ~3.4us otherwise).
"""

import os
from contextlib import ExitStack

import bass_rust
import numpy as np

import concourse.bass as bass
import concourse.tile as tile
from concourse import bacc, mybir
from concourse.bass_utils import run_bass_kernel_spmd
from concourse.masks import make_identity

E, DIN, H, DOUT = 16, 128, 512, 64
B_FULL = 8192
N_CORES = 8
HB = H // 128  # 4 h-blocks
F32 = mybir.dt.float32
BF = mybir.dt.bfloat16


def build_nc(bloc=B_FULL // N_CORES, nb=512, n_warm=36):
    nbt = bloc // nb  # batch blocks (outer loop)
    nt = nb // 128
    assert bloc % nb == 0 and nb % 128 == 0

    nc = bacc.Bacc("TRN2", target_bir_lowering=False, debug=False)

    x = nc.dram_tensor("x", [bloc, E, DIN], F32, kind="ExternalInput")
    W1 = nc.dram_tensor("W1", [E, DIN, H], F32, kind="ExternalInput")
    b1 = nc.dram_tensor("b1", [E, H], F32, kind="ExternalInput")
    W2 = nc.dram_tensor("W2", [E, H, H], F32, kind="ExternalInput")
    b2 = nc.dram_tensor("b2", [E, H], F32, kind="ExternalInput")
    W3 = nc.dram_tensor("W3", [E, H, DOUT], F32, kind="ExternalInput")
    b3 = nc.dram_tensor("b3", [E, DOUT], F32, kind="ExternalInput")
    out = nc.dram_tensor("out", [bloc, DOUT], F32, kind="ExternalOutput")
    xbf = nc.dram_tensor("xbf", [bloc, E, DIN], BF, kind="Internal")

    RELU = mybir.ActivationFunctionType.Relu
    ADD = mybir.AluOpType.add
    MAX = mybir.AluOpType.max

    with tile.TileContext(nc) as tc, ExitStack() as ctx:
        consts = ctx.enter_context(tc.tile_pool(name="consts", bufs=1))
        wall = ctx.enter_context(tc.tile_pool(name="wall", bufs=1))
        xtp = ctx.enter_context(tc.tile_pool(name="xtp", bufs=4))
        h1p = ctx.enter_context(tc.tile_pool(name="h1p", bufs=2))
        h2p = ctx.enter_context(tc.tile_pool(name="h2p", bufs=2))
        obp = ctx.enter_context(tc.tile_pool(name="obp", bufs=2))
        p1p = ctx.enter_context(tc.tile_pool(name="p1p", bufs=3, space="PSUM"))
        p2p = ctx.enter_context(tc.tile_pool(name="p2p", bufs=3, space="PSUM"))
        pop = ctx.enter_context(tc.tile_pool(name="pop", bufs=2, space="PSUM"))

        ident = consts.tile([128, 128], F32)
        make_identity(nc, ident)
        identr = consts.tile([128, 128], BF)
        nc.scalar.copy(identr, ident)

        # PE warmup: real (non-transpose) matmuls so the HAM clock gate sees
        # sustained activity and unthrottles 1.2 -> 2.4GHz before the first
        # data-dependent matmul is ready.
        pjunk = p1p.tile([128, nb], F32, tag="p1", name="junk")
        for _ in range(n_warm):
            nc.tensor.matmul(pjunk[:, :128], identr, identr, start=True, stop=True)

        # biases: load natural layout, PE-transpose so per-feature bias lands
        # on partitions: b1s[p, hb*E + e] = b1[e, hb*128 + p]
        b1n = consts.tile([E, H], F32)
        nc.sync.dma_start(out=b1n, in_=b1[:, :])
        b2n = consts.tile([E, H], F32)
        nc.sync.dma_start(out=b2n, in_=b2[:, :])
        b3n = consts.tile([E, DOUT], F32)
        nc.sync.dma_start(out=b3n, in_=b3[:, :])
        b1s = consts.tile([128, HB * E], F32)
        b2s = consts.tile([128, HB * E], F32)
        for bn, bs in ((b1n, b1s), (b2n, b2s)):
            pb = p2p.tile([128, HB * E], F32, tag="p2", name="pb")
            for hb in range(HB):
                nc.tensor.transpose(
                    pb[:, hb * E : (hb + 1) * E],
                    bn[:, hb * 128 : (hb + 1) * 128],
                    ident[:E, :E],
                )
            nc.vector.tensor_copy(bs, pb)
        pb3 = p2p.tile([DOUT, E], F32, tag="p2", name="pb3")
        nc.tensor.transpose(pb3, b3n, ident[:E, :E])
        b3s = consts.tile([DOUT, E], F32)
        nc.vector.tensor_copy(b3s, pb3)
        b3sum = consts.tile([DOUT, 1], F32)
        nc.vector.reduce_sum(b3sum, b3s, axis=bass_rust.AxisListType.X)

        # ---- weight + x-staging DMA emission (all on the gpsimd SWDGE
        # queue, cast fp32->bf16 in flight). x chunks are interleaved so the
        # first expert's x and weights land first. ----
        w1all = wall.tile([128, E, H], BF)
        w2all = wall.tile([128, E, HB, H], BF)
        w3all = wall.tile([128, E, HB, DOUT], BF)

        def stage_x(bt, g):
            # cast x[block, 4-expert group] into the bf16 DRAM mirror
            b0 = bt * nb
            nc.gpsimd.dma_start(
                out=xbf[b0 : b0 + nb, 4 * g : 4 * g + 4, :],
                in_=x[b0 : b0 + nb, 4 * g : 4 * g + 4, :],
            )

        def load_w(e0, ne):
            es = slice(e0, e0 + ne)
            nc.gpsimd.dma_start(
                out=w1all[:, es, :], in_=W1[es].rearrange("e d h -> d e h")
            )
            nc.gpsimd.dma_start(
                out=w2all[:, es],
                in_=W2[es].rearrange("e (hb p) g -> p e hb g", p=128),
            )
            nc.gpsimd.dma_start(
                out=w3all[:, es],
                in_=W3[es].rearrange("e (gb p) o -> p e gb o", p=128),
            )

        stage_x(0, 0)
        load_w(0, 1)
        stage_x(0, 1)
        load_w(1, 1)
        stage_x(0, 2)
        load_w(2, 2)
        stage_x(0, 3)
        load_w(4, 4)
        stage_x(1, 0)
        stage_x(1, 1)
        load_w(8, 4)
        stage_x(1, 2)
        stage_x(1, 3)
        load_w(12, 4)

        # ---- xt prefetch: XBAR transpose DMA (hardware DGE on the SP queue)
        # xt[d, j] = x[b0 + j, e, d], one DMA per (block, expert). ----
        xt_tiles = {}

        def emit_xt(idx):
            bt, e = divmod(idx, E)
            t = xtp.tile([128, nb], BF, tag="xt")
            b0 = bt * nb
            nc.sync.dma_start(out=t, in_=xbf[b0 : b0 + nb, e, :], transpose=True)
            xt_tiles[idx] = t

        for i in range(3):
            emit_xt(i)

        # ---- main loop: block-outer, expert-inner, L3 staggered one step ----
        pending = None  # (emit_l3_fn, po_tile_or_None_for_epilogue)

        def emit_epilogue(bt, po_t):
            b0 = bt * nb
            ob = obp.tile([DOUT, nb], F32, tag="ob")
            # ob = (po_lo + b3sum) + po_hi — two ops; the engine may read only
            # one PSUM operand per instruction
            nc.vector.tensor_scalar_add(ob, po_t[0:DOUT, :], b3sum)
            nc.vector.tensor_add(ob, ob, po_t[DOUT : 2 * DOUT, :])
            pot = pop.tile([128, nt * DOUT], F32, tag="po", name=f"pot{bt}")
            for t in range(nt):
                nc.tensor.transpose(
                    pot[:, t * DOUT : (t + 1) * DOUT],
                    ob[:, t * 128 : (t + 1) * 128],
                    ident[:DOUT, :DOUT],
                )
            obt = obp.tile([128, nt * DOUT], F32, tag="obt")
            nc.vector.tensor_copy(obt, pot)
            nc.sync.dma_start(
                out=out[b0 : b0 + nb, :].rearrange("(t p) o -> p t o", p=128),
                in_=obt.rearrange("p (t o) -> p t o", o=DOUT),
            )

        for bt in range(nbt):
            po_t = pop.tile([128, nb], F32, tag="po", name=f"po{bt}")
            for e in range(E):
                nxt = bt * E + e + 3
                if nxt < nbt * E:
                    emit_xt(nxt)
                xt = xt_tiles.pop(bt * E + e)

                # ---- layer 1 ----
                h1 = h1p.tile([128, HB, nb], BF, tag="h1")
                for hb in range(HB):
                    ps = p1p.tile([128, nb], F32, tag="p1")
                    nc.tensor.matmul(
                        ps,
                        w1all[:, e, hb * 128 : (hb + 1) * 128],
                        xt,
                        start=True,
                        stop=True,
                    )
                    bias = b1s[:, hb * E + e : hb * E + e + 1]
                    if hb % 2 == 0:
                        nc.scalar.activation(h1[:, hb, :], ps, RELU, bias=bias)
                    else:
                        nc.vector.tensor_scalar(h1[:, hb, :], ps, bias, 0.0, ADD, MAX)

                # staggered L3 from the previous step (deps are met by now)
                if pending is not None:
                    fn, ep = pending
                    fn()
                    if ep is not None:
                        emit_epilogue(*ep)
                    pending = None

                # ---- layer 2 ----
                h2 = h2p.tile([128, HB, nb], BF, tag="h2")
                for gb in range(HB):
                    ps = p2p.tile([128, nb], F32, tag="p2")
                    for hb in range(HB):
                        nc.tensor.matmul(
                            ps,
                            w2all[:, e, hb, gb * 128 : (gb + 1) * 128],
                            h1[:, hb, :],
                            start=(hb == 0),
                            stop=(hb == HB - 1),
                        )
                    bias = b2s[:, gb * E + e : gb * E + e + 1]
                    if gb % 2 == 0:
                        nc.scalar.activation(h2[:, gb, :], ps, RELU, bias=bias)
                    else:
                        nc.vector.tensor_scalar(h2[:, gb, :], ps, bias, 0.0, ADD, MAX)

                # ---- layer 3 (column-tiled, staggered emission) ----
                def mk_l3(e=e, h2=h2, po_t=po_t):
                    def go():
                        for gb in range(HB):
                            half = (gb % 2) * DOUT
                            # two interleaved accumulation groups share the
                            # bank (column halves); per-element has_written
                            # bits keep this correct on HW
                            nc.tensor.matmul(
                                po_t[half : half + DOUT, :],
                                w3all[:, e, gb, :],
                                h2[:, gb, :],
                                start=(e == 0 and gb < 2),
                                stop=(e == E - 1 and gb >= 2),
                                skip_group_check=True,
                            )

                    return go

                pending = (mk_l3(), (bt, po_t) if e == E - 1 else None)

        fn, ep = pending
        fn()
        emit_epilogue(*ep)

    nc.compile()
    return nc


_NC_CACHE = {}


def _get_nc():
    n_warm = int(os.environ.get("KERNEL_WARM", "36"))
    if n_warm not in _NC_CACHE:
        _NC_CACHE[n_warm] = build_nc(n_warm=n_warm)
    return _NC_CACHE[n_warm]


def kernel(x, W1, b1, W2, b2, W3, b3):
    x = np.ascontiguousarray(np.asarray(x, dtype=np.float32))
    ws = {
        "W1": np.ascontiguousarray(np.asarray(W1, dtype=np.float32)),
        "b1": np.ascontiguousarray(np.asarray(b1, dtype=np.float32)),
        "W2": np.ascontiguousarray(np.asarray(W2, dtype=np.float32)),
        "b2": np.ascontiguousarray(np.asarray(b2, dtype=np.float32)),
        "W3": np.ascontiguousarray(np.asarray(W3, dtype=np.float32)),
        "b3": np.ascontiguousarray(np.asarray(b3, dtype=np.float32)),
    }
    nc = _get_nc()
    shards = np.split(x, N_CORES, axis=0)
    in_maps = [{"x": np.ascontiguousarray(s), **ws} for s in shards]
    trace = bool(int(os.environ.get("KERNEL_TRACE", "0")))
    kwargs = {}
    if trace and os.environ.get("KERNEL_TRACE_DIR"):
        kwargs["tmpdir"] = os.environ["KERNEL_TRACE_DIR"]
    res = run_bass_kernel_spmd(nc, in_maps, list(range(N_CORES)), trace=trace, **kwargs)
    if trace:
        kernel.last_results = res
    return np.concatenate([res.results[c]["out"] for c in range(N_CORES)], axis=0)


# revision 7
# speedup vs baseline: 1.4438x; 1.4438x over previous
"""Trainium2 Bass kernel for grouped-expert 3-layer MLP (MoE, known covariance).

Computes, for x[B, E, DIN] and per-expert weights:
    h1 = relu(x[:,e] @ W1[e] + b1[e])      # [B, H]
    h2 = relu(h1 @ W2[e] + b2[e])          # [B, H]
    o  = h2 @ W3[e] + b3[e]                # [B, DOUT]
    out = sum_e o                          # [B, DOUT]

Sharding: data-parallel over batch across 8 NeuronCores (B=8192 -> 1024/core).
Weights are replicated to every core; no collectives needed.

Per-core schedule (v3):
  - All matmuls run in bf16 (1 cycle/row on the PE; fp8 would be 2x but its
    quantization noise measures ~3-6e-2 on this problem vs the 2e-2 gate).
    Inputs are cast fp32->bf16 in-flight by the gpsimd SWDGE DMA.
  - Expert-outer loop with double-buffered weights: the 23MB weight stream is
    spread over the whole kernel (~150GB/s); a block-outer variant needs 2x
    that and starves the PE (measured).
  - x tiles are PE-transposed to feature-major in bf16 (1 cyc/row). The
    transposes for expert e+1 are emitted between L1(e) and L2(e) so their
    PSUM->SBUF evacuation is complete long before L1(e+1) consumes them.
  - Layer 3 (M=DOUT=64) runs column-tiled: gb even -> PSUM partitions 0:64,
    gb odd -> 64:128, two concurrent matmuls in separate PE column groups,
    accumulated over all experts; the halves are summed in the epilogue.
  - L3 of expert e is emitted after L1 of expert e+1 (software stagger) so its
    h2-evacuation dependency is met without stalling the PE.
  - A short burst of identity matmuls at t=0 warms the PE HAM clock gate
    (cold PE runs at 1.2GHz for the first ~3.4us otherwise).
"""

import os
from contextlib import ExitStack

import bass_rust
import numpy as np

import concourse.bass as bass
import concourse.tile as tile
from concourse import bacc, mybir
from concourse.bass_utils import run_bass_kernel_spmd
from concourse.masks import make_identity

E, DIN, H, DOUT = 16, 128, 512, 64
B_FULL = 8192
N_CORES = 8
HB = H // 128  # 4 h-blocks
F32 = mybir.dt.float32
BF = mybir.dt.bfloat16


def build_nc(bloc=B_FULL // N_CORES, nb=512, n_warm=36):
    nbt = bloc // nb  # batch tiles per core
    nt = nb // 128
    assert bloc % nb == 0 and nb % 128 == 0

    nc = bacc.Bacc("TRN2", target_bir_lowering=False, debug=False)

    x = nc.dram_tensor("x", [bloc, E, DIN], F32, kind="ExternalInput")
    W1 = nc.dram_tensor("W1", [E, DIN, H], F32, kind="ExternalInput")
    b1 = nc.dram_tensor("b1", [E, H], F32, kind="ExternalInput")
    W2 = nc.dram_tensor("W2", [E, H, H], F32, kind="ExternalInput")
    b2 = nc.dram_tensor("b2", [E, H], F32, kind="ExternalInput")
    W3 = nc.dram_tensor("W3", [E, H, DOUT], F32, kind="ExternalInput")
    b3 = nc.dram_tensor("b3", [E, DOUT], F32, kind="ExternalInput")
    out = nc.dram_tensor("out", [bloc, DOUT], F32, kind="ExternalOutput")

    RELU = mybir.ActivationFunctionType.Relu
    ADD = mybir.AluOpType.add
    MAX = mybir.AluOpType.max

    with tile.TileContext(nc) as tc, ExitStack() as ctx:
        consts = ctx.enter_context(tc.tile_pool(name="consts", bufs=1))
        w1p = ctx.enter_context(tc.tile_pool(name="w1p", bufs=2))
        w2p = ctx.enter_context(tc.tile_pool(name="w2p", bufs=2))
        w3p = ctx.enter_context(tc.tile_pool(name="w3p", bufs=2))
        xp = ctx.enter_context(tc.tile_pool(name="xp", bufs=4))
        xtp = ctx.enter_context(tc.tile_pool(name="xtp", bufs=4))
        h1p = ctx.enter_context(tc.tile_pool(name="h1p", bufs=2))
        h2p = ctx.enter_context(tc.tile_pool(name="h2p", bufs=2))
        obp = ctx.enter_context(tc.tile_pool(name="obp", bufs=2))
        p1p = ctx.enter_context(tc.tile_pool(name="p1p", bufs=3, space="PSUM"))
        p2p = ctx.enter_context(tc.tile_pool(name="p2p", bufs=3, space="PSUM"))
        pop = ctx.enter_context(tc.tile_pool(name="pop", bufs=2, space="PSUM"))

        ident = consts.tile([128, 128], F32)
        make_identity(nc, ident)
        identr = consts.tile([128, 128], BF)
        nc.scalar.copy(identr, ident)

        # PE warmup: real (non-transpose) matmuls so the HAM clock gate sees
        # sustained activity and unthrottles 1.2 -> 2.4GHz before the first
        # data-dependent matmul issues.
        pjunk = p1p.tile([128, nb], F32, tag="p1", name="junk")
        for _ in range(n_warm):
            nc.tensor.matmul(pjunk[:, :128], identr, identr, start=True, stop=True)

        # biases: load natural layout, PE-transpose so the per-feature bias
        # lands on partitions: b1s[p, hb*E + e] = b1[e, hb*128 + p]
        b1n = consts.tile([E, H], F32)
        nc.sync.dma_start(out=b1n, in_=b1[:, :])
        b2n = consts.tile([E, H], F32)
        nc.sync.dma_start(out=b2n, in_=b2[:, :])
        b3n = consts.tile([E, DOUT], F32)
        nc.sync.dma_start(out=b3n, in_=b3[:, :])
        b1s = consts.tile([128, HB * E], F32)
        b2s = consts.tile([128, HB * E], F32)
        for bn, bs in ((b1n, b1s), (b2n, b2s)):
            pb = p2p.tile([128, HB * E], F32, tag="p2", name="pb")
            for hb in range(HB):
                nc.tensor.transpose(
                    pb[:, hb * E : (hb + 1) * E],
                    bn[:, hb * 128 : (hb + 1) * 128],
                    ident[:E, :E],
                )
            nc.vector.tensor_copy(bs, pb)
        pb3 = p2p.tile([DOUT, E], F32, tag="p2", name="pb3")
        nc.tensor.transpose(pb3, b3n, ident[:E, :E])
        b3s = consts.tile([DOUT, E], F32)
        nc.vector.tensor_copy(b3s, pb3)
        b3sum = consts.tile([DOUT, 1], F32)
        nc.vector.reduce_sum(b3sum, b3s, axis=bass_rust.AxisListType.X)

        # ---- DMA emission helpers (gpsimd SWDGE, cast fp32->bf16) ----
        xin_tiles = {}

        def load_x(e):
            for bt in range(nbt):
                b0 = bt * nb
                t = xp.tile([128, nt, DIN], BF, tag="xin")
                nc.gpsimd.dma_start(
                    out=t,
                    in_=x[b0 : b0 + nb, e, :].rearrange("(t p) d -> p t d", p=128),
                )
                xin_tiles[(e, bt)] = t

        w_tiles = {}

        def load_w(e):
            w1t = w1p.tile([DIN, H], BF, tag="w1")
            nc.gpsimd.dma_start(out=w1t, in_=W1[e])
            w2t = w2p.tile([128, HB, H], BF, tag="w2")
            nc.gpsimd.dma_start(
                out=w2t, in_=W2[e].rearrange("(hb p) g -> p hb g", p=128)
            )
            w3t = w3p.tile([128, HB, DOUT], BF, tag="w3")
            nc.gpsimd.dma_start(
                out=w3t, in_=W3[e].rearrange("(gb p) o -> p gb o", p=128)
            )
            w_tiles[e] = (w1t, w2t, w3t)

        # bootstrap loads: x first so the first transposes aren't stuck
        # behind the big weight transfers
        load_x(0)
        load_w(0)
        load_x(1)
        load_w(1)

        xt_tiles = {}

        def emit_transposes(e):
            # PE transpose xin -> feature-major xt, both batch tiles
            for bt in range(nbt):
                xin = xin_tiles.pop((e, bt))
                pxt = p1p.tile([DIN, nb], BF, tag="p1", name="pxt")
                for t in range(nt):
                    nc.tensor.transpose(
                        pxt[:, t * 128 : (t + 1) * 128], xin[:, t, :], identr
                    )
                xt = xtp.tile([DIN, nb], BF, tag="xt")
                if bt % 2 == 0:
                    nc.scalar.copy(xt, pxt)
                else:
                    nc.vector.tensor_copy(xt, pxt)
                xt_tiles[(e, bt)] = xt

        emit_transposes(0)

        pending = None  # staggered L3 emitter from the previous expert

        for e in range(E):
            if e + 2 < E:
                load_x(e + 2)
            if e + 1 < E:
                load_w(e + 1)
            w1t, w2t, w3t = w_tiles.pop(e)

            # ---- layer 1, both batch tiles ----
            h1 = [h1p.tile([128, HB, nb], BF, tag=f"h1_{bt}", name=f"h1_{bt}") for bt in range(nbt)]
            for bt in range(nbt):
                xt = xt_tiles.pop((e, bt))
                for hb in range(HB):
                    ps = p1p.tile([128, nb], F32, tag="p1")
                    nc.tensor.matmul(
                        ps, w1t[:, hb * 128 : (hb + 1) * 128], xt, start=True, stop=True
                    )
                    bias = b1s[:, hb * E + e : hb * E + e + 1]
                    if (bt + hb) % 2 == 0:
                        nc.scalar.activation(h1[bt][:, hb, :], ps, RELU, bias=bias)
                    else:
                        nc.vector.tensor_scalar(
                            h1[bt][:, hb, :], ps, bias, 0.0, ADD, MAX
                        )

            # staggered L3 from the previous expert (h2 evacs done by now)
            if pending is not None:
                pending()
                pending = None

            # transposes for the next expert (consumed 30+ matmuls later)
            if e + 1 < E:
                emit_transposes(e + 1)

            # ---- layer 2, both batch tiles ----
            h2 = [h2p.tile([128, HB, nb], BF, tag=f"h2_{bt}", name=f"h2_{bt}") for bt in range(nbt)]
            for bt in range(nbt):
                for gb in range(HB):
                    ps = p2p.tile([128, nb], F32, tag="p2")
                    for hb in range(HB):
                        nc.tensor.matmul(
                            ps,
                            w2t[:, hb, gb * 128 : (gb + 1) * 128],
                            h1[bt][:, hb, :],
                            start=(hb == 0),
                            stop=(hb == HB - 1),
                        )
                    bias = b2s[:, gb * E + e : gb * E + e + 1]
                    if (bt + gb) % 2 == 0:
                        nc.scalar.activation(h2[bt][:, gb, :], ps, RELU, bias=bias)
                    else:
                        nc.vector.tensor_scalar(
                            h2[bt][:, gb, :], ps, bias, 0.0, ADD, MAX
                        )

            # ---- layer 3: column-tiled accumulation into po, staggered ----
            def mk_l3(e=e, h2=h2, w3t=w3t):
                def go():
                    for bt in range(nbt):
                        for gb in range(HB):
                            half = (gb % 2) * DOUT
                            # two interleaved accumulation groups share the
                            # bank (column halves); per-element has_written
                            # bits keep this correct on HW
                            nc.tensor.matmul(
                                po[bt][half : half + DOUT, :],
                                w3t[:, gb, :],
                                h2[bt][:, gb, :],
                                start=(e == 0 and gb < 2),
                                stop=(e == E - 1 and gb >= 2),
                                skip_group_check=True,
                            )

                return go

            if e == 0:
                po = [
                    pop.tile([128, nb], F32, tag="po", name=f"po{bt}")
                    for bt in range(nbt)
                ]
            pending = mk_l3()

        pending()

        # ---- epilogue: sum halves + bias, transpose to batch-major, store ----
        for bt in range(nbt):
            b0 = bt * nb
            ob = obp.tile([DOUT, nb], F32, tag="ob")
            # ob = (po_lo + b3sum) + po_hi — two ops; the engine may read only
            # one PSUM operand per instruction
            nc.vector.tensor_scalar_add(ob, po[bt][0:DOUT, :], b3sum)
            nc.vector.tensor_add(ob, ob, po[bt][DOUT : 2 * DOUT, :])
            pot = pop.tile([128, nt * DOUT], F32, tag="po", name=f"pot{bt}")
            for t in range(nt):
                nc.tensor.transpose(
                    pot[:, t * DOUT : (t + 1) * DOUT],
                    ob[:, t * 128 : (t + 1) * 128],
                    ident[:DOUT, :DOUT],
                )
            obt = obp.tile([128, nt * DOUT], F32, tag="obt")
            nc.vector.tensor_copy(obt, pot)
            nc.sync.dma_start(
                out=out[b0 : b0 + nb, :].rearrange("(t p) o -> p t o", p=128),
                in_=obt.rearrange("p (t o) -> p t o", o=DOUT),
            )

    nc.compile()
    return nc


_NC_CACHE = {}


def _get_nc():
    n_warm = int(os.environ.get("KERNEL_WARM", "36"))
    if n_warm not in _NC_CACHE:
        _NC_CACHE[n_warm] = build_nc(n_warm=n_warm)
    return _NC_CACHE[n_warm]


def kernel(x, W1, b1, W2, b2, W3, b3):
    x = np.ascontiguousarray(np.asarray(x, dtype=np.float32))
    ws = {
        "W1": np.ascontiguousarray(np.asarray(W1, dtype=np.float32)),
        "b1": np.ascontiguousarray(np.asarray(b1, dtype=np.float32)),
        "W2": np.ascontiguousarray(np.asarray(W2, dtype=np.float32)),
        "b2": np.ascontiguousarray(np.asarray(b2, dtype=np.float32)),
        "W3": np.ascontiguousarray(np.asarray(W3, dtype=np.float32)),
        "b3": np.ascontiguousarray(np.asarray(b3, dtype=np.float32)),
    }
    nc = _get_nc()
    shards = np.split(x, N_CORES, axis=0)
    in_maps = [{"x": np.ascontiguousarray(s), **ws} for s in shards]
    trace = bool(int(os.environ.get("KERNEL_TRACE", "0")))
    kwargs = {}
    if trace and os.environ.get("KERNEL_TRACE_DIR"):
        kwargs["tmpdir"] = os.environ["KERNEL_TRACE_DIR"]
    res = run_bass_kernel_spmd(nc, in_maps, list(range(N_CORES)), trace=trace, **kwargs)
    if trace:
        kernel.last_results = res
    return np.concatenate([res.results[c]["out"] for c in range(N_CORES)], axis=0)
